# revision 25
# baseline (speedup 1.0000x reference)
"""Causal self-attention on 8 Trainium2 NeuronCores.

Sharding: core = (batch b in {0,1}) x (head-group g in {0..3}), 4 heads per
core. Each core computes qkv for its heads from x[b], runs causal attention,
and multiplies by its 256 rows of w_proj, producing a partial [T, C] output.
Host sums the 4 partials per batch.

Layout: everything is computed "transposed" so no on-chip transposes are
needed. The host feeds x[b].T; q^T/k^T come out of the qkv matmul with
head-dim on partitions (exactly the S^T = K Q^T operand layout); softmax is
done on S^T (keys on partitions, queries on free) with the denominator
obtained by appending a ones-column to V in the A@V matmul; the A@V output
Y^T is exactly the lhsT layout the final projection needs.

v5 structure (v4 178us, v3 171.4us, baseline 222.9us):
- Input DMA prioritized: only the bytes pass0 needs (xT + the q0/k0
  halves of wqk, 4.5MB) go first; wv/mask/q1k1/wp ride behind.  The
  whole early phase is HBM-bandwidth-gated, so this directly pulls the
  first exp earlier.
- PE warmup burst (~3.6us) so HAM is at 8/8 when pass0 starts; pass0
  keeps ~100% PE duty against the DMA stream (8 matmuls per c-tile).
- Pair-major attention; stage-B tiles 0-3 inline in qc0; pass1 and
  B4-15 drip as PE filler units; per-pair normalization.
- Tail: all reserve projection groups are emitted BEFORE the
  normalization chain (the PE queue is in-order - fillers emitted after
  a blocked instruction never fill anything), sums DMAs issued before
  the ytr shifts, and the last groups alternate PSUM pools and
  eviction engines to pipeline 4 deep.
"""

import numpy as np
import ml_dtypes

import concourse.bass as bass
import concourse.bacc as bacc
import concourse.tile as tile
from concourse import mybir
from concourse.bass_utils import run_bass_kernel_spmd

F32 = mybir.dt.float32
BF16 = mybir.dt.bfloat16
EXP = mybir.ActivationFunctionType.Exp

B, T, C, H, HD = 2, 2048, 1024, 16, 64
NCORES = 8
HPC = 4      # heads per core
NPAIR = 2    # head pairs per core
NCT = C // 128   # 8 c-tiles
NTT = T // 128   # 16 t-tiles
NQC = T // 512   # 4 query chunks
SCALE = 1.0 / np.sqrt(HD)
NEG = -1.0e30


def build_kernel():
    nc = bacc.Bacc("TRN2", target_bir_lowering=False, debug=False, num_devices=NCORES)

    xT = nc.dram_tensor("xT", [C, T], BF16, kind="ExternalInput")
    wqk = nc.dram_tensor("wqk", [C, 512], BF16, kind="ExternalInput")
    wv = nc.dram_tensor("wv", [C, 256], BF16, kind="ExternalInput")
    wp = nc.dram_tensor("wp", [256, C], BF16, kind="ExternalInput")
    maskc = nc.dram_tensor("maskc", [128, 256], BF16, kind="ExternalInput")
    sel = nc.dram_tensor("sel", [2, 128], BF16, kind="ExternalInput")
    out = nc.dram_tensor("out", [T, C], BF16, kind="ExternalOutput")

    with tile.TileContext(nc) as tc:
        _body(tc, xT, wqk, wv, wp, maskc, sel, out)

    nc.compile()
    return nc


def _body(tc, xT, wqk, wv, wp, maskc, sel, out):
    nc = tc.nc
    from contextlib import ExitStack

    with ExitStack() as ctx:
        sb = lambda name: ctx.enter_context(tc.tile_pool(name=name, bufs=1))
        qkT_sb = sb("qkT").tile([128, 4 * T], BF16)       # bands q0,k0,q1,k1
        v65_sb = sb("v65").tile([128, NTT * 260], BF16)   # per k-tile: 4x(64 v + 1 ones)
        yt_sb = sb("yt").tile([128, NPAIR * T], BF16)     # pair p: rows 0-63 head 2p, 64-127 head 2p+1
        wp_sb = sb("wp").tile([128, 2 * C], BF16)
        maskc_sb = sb("maskc").tile([128, 256], BF16)
        sel_sb = sb("sel").tile([2, 128], BF16)
        wu_sb = sb("wu").tile([128, 128], BF16)

        es_pool = ctx.enter_context(tc.tile_pool(name="es", bufs=4))
        sums_pool = ctx.enter_context(tc.tile_pool(name="sums", bufs=2))
        rc_pool = ctx.enter_context(tc.tile_pool(name="rc", bufs=2))
        avst_pool = ctx.enter_context(tc.tile_pool(name="avst", bufs=2))
        ytr_pool = ctx.enter_context(tc.tile_pool(name="ytr", bufs=2))
        ost_pool = ctx.enter_context(tc.tile_pool(name="ost", bufs=3))

        # PSUM: psS tag = 2 slots x [128,1024] (4 banks), av 2 banks, misc 2
        ps = ctx.enter_context(tc.tile_pool(name="ps", bufs=2, space="PSUM"))
        av_pool = ctx.enter_context(tc.tile_pool(name="av", bufs=2, space="PSUM"))
        psS_pool = ctx.enter_context(tc.tile_pool(name="psS", bufs=2, space="PSUM"))

        xw_pool = ctx.enter_context(tc.tile_pool(name="xw", bufs=1))
        xT_sb = xw_pool.tile([128, NCT * T], BF16, name="xT_sb")
        wqk_sb = xw_pool.tile([128, NCT * 512], BF16, name="wqk_sb")
        wv_sb = xw_pool.tile([128, NCT * 256], BF16, name="wv_sb")

        # loads: ONLY what pass0 needs first (bands q0,k0 = first 256 cols of
        # each wqk c-tile + xT) - the early phase is HBM-bandwidth-gated.
        for c in range(NCT):
            nc.sync.dma_start(wqk_sb[:, c * 512:c * 512 + 256], wqk[c * 128:(c + 1) * 128, 0:256])
            if c % 2 == 0:  # 1MB c-pair tiles: near the DMA bandwidth knee
                nc.sync.dma_start(
                    xT_sb[:, c * T:(c + 2) * T].rearrange("p (c t) -> p c t", c=2),
                    xT.ap().rearrange("(c p) t -> p c t", c=NCT)[:, c:c + 2, :])
        nc.sync.dma_start(
            wv_sb[:].rearrange("p (c j) -> p c j", c=NCT),
            wv.ap().rearrange("(c p) j -> p c j", c=NCT))
        nc.sync.dma_start(maskc_sb[:], maskc[:])
        nc.sync.dma_start(sel_sb[:], sel[:])
        nc.sync.dma_start(  # bands q1,k1 (cols 256:512 of each c-tile)
            wqk_sb[:].rearrange("p (c j) -> p c j", c=NCT)[:, :, 256:512],
            wqk.ap().rearrange("(c p) j -> p c j", c=NCT)[:, :, 256:512])
        nc.sync.dma_start(
            wp_sb[:].rearrange("p (q c) -> p q c", q=2),
            wp.ap().rearrange("(q p) c -> p q c", q=2))
        v65_4d = v65_sb[:].rearrange("p (t h d) -> p t h d", t=NTT, h=HPC, d=65)
        nc.vector.memset(v65_4d[:, :, :, 64:65], 1.0)
        nc.vector.memset(wu_sb[:], 0.0)

        # ---- PE warmup: ~3.6us of back-to-back dummy matmuls while the
        # first input DMAs land, so HAM un-throttles the PE clock.
        wu_ps = psS_pool.tile([128, 1024], F32, tag="psS", name="warmup")
        for i in range(34):
            nc.tensor.matmul(wu_ps[:, 0:128], wu_sb[:], wu_sb[:], start=True, stop=True)

        # ---- stage A pass0: bands q0,k0 (all four t-chunks), c-outer so
        # each xT c-tile feeds its 8 matmuls the moment the DMA lands;
        # PE duty stays ~100% against the DMA stream, keeping HAM warm.
        accs = {}
        for b in (0, 1):
            acc01 = psS_pool.tile([128, 1024], F32, tag="psS", name=f"accA_{b}_01")
            acc2 = av_pool.tile([128, 512], F32, tag="av", name=f"accA_{b}_2")
            acc3 = ps.tile([128, 512], F32, tag="ps", name=f"accA_{b}_3")
            accs[b] = [acc01, acc2, acc3]
        for c in range(NCT):
            for b in (0, 1):
                lhs = wqk_sb[:, c * 512 + b * 128: c * 512 + (b + 1) * 128]
                acc01, acc2, acc3 = accs[b]
                dsts = [acc01[:, 0:512], acc01[:, 512:1024], acc2[:], acc3[:]]
                for t4 in range(4):
                    nc.tensor.matmul(
                        dsts[t4], lhs,
                        xT_sb[:, c * T + t4 * 512: c * T + (t4 + 1) * 512],
                        start=(c == 0), stop=(c == NCT - 1))
        for b in (0, 1):
            acc01, acc2, acc3 = accs[b]
            nc.vector.tensor_copy(qkT_sb[:, b * T: b * T + 1024], acc01[:])
            nc.vector.tensor_copy(qkT_sb[:, b * T + 1024: b * T + 1536], acc2[:])
            nc.vector.tensor_copy(qkT_sb[:, b * T + 1536: b * T + 2048], acc3[:])

        # ---- stage A pass1 (bands q1,k1): single-bank chunks over the 8
        # c-tiles, emitted as two 4-matmul pending units (PE filler).
        def make_A_chunk(b, t4):
            state = {}

            def half1():
                state["acc"] = ps.tile([128, 512], F32, tag="ps", name=f"accA_{b}_{t4}")
                for c in range(4):
                    nc.tensor.matmul(state["acc"][:],
                                     wqk_sb[:, c * 512 + b * 128: c * 512 + (b + 1) * 128],
                                     xT_sb[:, c * T + t4 * 512: c * T + (t4 + 1) * 512],
                                     start=(c == 0), stop=False)

            def half2():
                for c in range(4, NCT):
                    nc.tensor.matmul(state["acc"][:],
                                     wqk_sb[:, c * 512 + b * 128: c * 512 + (b + 1) * 128],
                                     xT_sb[:, c * T + t4 * 512: c * T + (t4 + 1) * 512],
                                     start=False, stop=(c == NCT - 1))
                nc.vector.tensor_copy(qkT_sb[:, b * T + t4 * 512: b * T + (t4 + 1) * 512],
                                      state["acc"][:])

            return [half1, half2]

        # ---- stage B: v natural [t, j]
        def emit_B(t):
            psv = ps.tile([128, 512], F32, tag="ps", name=f"psv_{t}")
            for c in range(NCT):
                lhs = xT_sb[:, c * T + t * 128: c * T + (t + 1) * 128]
                nc.tensor.matmul(psv[:, 0:256], lhs, wv_sb[:, c * 256:(c + 1) * 256],
                                 start=(c == 0), stop=(c == NCT - 1))
            dst = v65_sb[:, t * 260:(t + 1) * 260].rearrange("p (h d) -> p h d", h=HPC, d=65)
            src_ = psv[:, 0:256].rearrange("p (h d) -> p h d", h=HPC, d=64)
            nc.vector.tensor_copy(dst[:, :, 0:64], src_)

        # ---- stage D: projection (needs both pairs' normalized Y^T)
        def emit_proj_group(t, n, ev="v", pool=None):
            pool = pool or ps
            pso = pool.tile([128, 512] if pool is ps else [128, 1024], F32,
                            tag="ps" if pool is ps else "psS", name=f"pso_{t}_{n}")
            for p in range(NPAIR):
                lhsT = yt_sb[:, p * T + t * 128: p * T + (t + 1) * 128]
                rhs = wp_sb[:, p * C + n * 512: p * C + (n + 1) * 512]
                nc.tensor.matmul(pso[:, 0:512], lhsT, rhs, start=(p == 0), stop=(p == NPAIR - 1))
            ost = ost_pool.tile([128, 512], BF16, tag="ost", name=f"ost_{t}_{n}")
            if ev == "s":
                nc.scalar.copy(ost[:], pso[:, 0:512])
            else:
                nc.vector.tensor_copy(ost[:], pso[:, 0:512])
            nc.sync.dma_start(out[t * 128:(t + 1) * 128, n * 512:(n + 1) * 512], ost[:])

        pending = []
        for t4 in (0, 1):
            pending += make_A_chunk(2, t4) + make_A_chunk(3, t4)
            pending += [lambda t=t: emit_B(t) for t in range(4 + 6 * t4, 10 + 6 * t4)]
        # pass1's t2/t3 chunks are only needed by pair1's qc2/qc3 - hold
        # them back to fill pair1's drip-dry early chunks (otherwise the
        # pair transition idles the PE past the HAM window).
        late = []
        for t4 in (2, 3):
            late += make_A_chunk(2, t4) + make_A_chunk(3, t4)
        reserve = []  # drained right before the tail normalization chain

        def run_pair(p):
            qb, kb = 2 * p, 2 * p + 1
            for qc in range(NQC):
                nkt = 4 * qc + 4
                av = [av_pool.tile([128, 512], F32, tag="av", name=f"av_{p}_{qc}_{i}") for i in range(2)]

                def emit_S(kt, qc=qc):
                    psb = psS_pool.tile([128, 1024], F32, tag="psS", name=f"psS_{p}_{qc}_{kt}")
                    slo = max(kt - 4 * qc, 0) * 128
                    for h in range(2):
                        base = 64 * h
                        lhsT = qkT_sb[base:base + 64, kb * T + kt * 128: kb * T + (kt + 1) * 128]
                        rhs = qkT_sb[base:base + 64, qb * T + qc * 512 + slo: qb * T + (qc + 1) * 512]
                        nc.tensor.matmul(psb[:, h * 512 + slo:(h + 1) * 512], lhsT, rhs,
                                         start=True, stop=True, tile_position=(base, 0))
                    return psb

                pipe = [emit_S(0)]
                if nkt > 1:
                    pipe.append(emit_S(1))
                for kt in range(nkt):
                    cur = pipe.pop(0)
                    if kt + 2 < nkt:
                        pipe.append(emit_S(kt + 2))
                    d = kt - 4 * qc
                    lo = max(d, 0) * 128  # first valid query column of this k-tile
                    psb2 = cur[:].rearrange("p (h q) -> p h q", h=2, q=512)
                    es = es_pool.tile([128, 1024], BF16, tag="es", name=f"es_{p}_{qc}_{kt}")
                    es2 = es[:].rearrange("p (h q) -> p h q", h=2, q=512)
                    nc.scalar.activation(es2[:, :, lo:], psb2[:, :, lo:], EXP, scale=SCALE)
                    if d >= 0:
                        # causal mask: zero the upper triangle of the diag
                        # block AFTER exp (bf16 multiply, off the S->exp
                        # critical path; the ones-column denominators sum
                        # es after this, so they stay exact)
                        nc.vector.tensor_mul(
                            es2[:, :, lo:lo + 128], es2[:, :, lo:lo + 128],
                            maskc_sb[:].rearrange("p (h q) -> p h q", h=2, q=128))
                    if p == 0 and qc == 0:
                        emit_B(kt)  # v tiles 0-3, needed by this chunk's AV
                    for h in range(2):
                        hh = 2 * p + h
                        lhsT_v = v65_sb[:, kt * 260 + hh * 65: kt * 260 + (hh + 1) * 65]
                        nc.tensor.matmul(av[h][0:65, lo:], lhsT_v, es[:, h * 512 + lo:(h + 1) * 512],
                                         start=(kt == 0), stop=(kt == nkt - 1))
                    popped = 0
                    lim = 2 if len(pending) > 8 else 1
                    while pending and popped < lim:
                        pending.pop(0)()
                        popped += 1
                # evict Y^T + sums; sums DMAs go FIRST (they gate the
                # reciprocal chain), the ytr partition-shift DMAs after.
                ytr = ytr_pool.tile([128, 512], F32, tag="ytr", name=f"ytr_{p}_{qc}")
                sums2 = sums_pool.tile([2, 512], F32, tag="sums", name=f"sums_{p}_{qc}")
                sts = []
                tail_qc = p == 1 and qc == NQC - 1
                for h in range(2):
                    st = avst_pool.tile([65, 512], F32, tag="avst", name=f"avst_{p}_{qc}_{h}")
                    if tail_qc:
                        nc.scalar.copy(st[:], av[h][0:65, :])  # ACT is idle post-exp
                    else:
                        nc.vector.tensor_copy(st[:], av[h][0:65, :])
                    nc.sync.dma_start(sums2[h:h + 1, :], st[64:65, :])
                    sts.append(st)
                for h in range(2):
                    nc.sync.dma_start(ytr[64 * h:64 * (h + 1), :], sts[h][0:64, :])

                def norm_pair(qc=qc, ytr=ytr, sums2=sums2):
                    rcf = rc_pool.tile([2, 512], F32, tag="rcf", name=f"rcf_{p}_{qc}")
                    rc2 = rc_pool.tile([2, 512], BF16, tag="rc", name=f"rc_{p}_{qc}")
                    nc.vector.reciprocal_approx_fast(rcf[:], sums2[:])
                    nc.vector.tensor_copy(rc2[:], rcf[:])
                    psR = ps.tile([128, 512], F32, tag="ps", name=f"psR_{p}_{qc}")
                    nc.tensor.matmul(psR[:], sel_sb[:], rc2[:], start=True, stop=True)
                    nc.vector.tensor_mul(yt_sb[:, p * T + qc * 512: p * T + (qc + 1) * 512],
                                         ytr[:], psR[:])

                if p == 1 and qc == NQC - 1:
                    # tail: ALL reserve groups first (in-order PE queue -
                    # they fill the normalization chain's latency), then the
                    # last 8 groups 4-deep across two PSUM pools with
                    # alternating eviction engines.
                    while reserve:
                        reserve.pop(0)()
                    norm_pair()
                    for i, (t, n) in enumerate([(t, n) for t in range(12, 16) for n in range(2)]):
                        emit_proj_group(t, n, ev="sv"[i % 2], pool=(ps, psS_pool)[i % 2])
                else:
                    pending.append(norm_pair)
                if p == 1 and qc < NQC - 1:
                    items = [lambda t=t, n=n: emit_proj_group(t, n)
                             for t in range(4 * qc, 4 * qc + 4) for n in range(2)]
                    if qc == NQC - 2:
                        pending.extend(items[:3])
                        reserve.extend(items[3:])
                    elif qc == NQC - 3:
                        pending.extend(items[:5])
                        reserve.extend(items[5:])
                    else:
                        pending.extend(items)

        run_pair(0)
        pending[0:0] = late
        run_pair(1)
        for fn in pending:
            fn()


_NC_CACHE = None


def _get_nc():
    global _NC_CACHE
    if _NC_CACHE is None:
        _NC_CACHE = build_kernel()
    return _NC_CACHE


def _make_in_maps(x, w_attn, w_proj):
    bf16 = ml_dtypes.bfloat16
    x = np.asarray(x, dtype=np.float32)
    w_attn = np.asarray(w_attn, dtype=np.float32)
    w_proj = np.asarray(w_proj, dtype=np.float32)
    # maskc: [128, 2x128] 0/1 keep-mask for the diagonal blocks
    # (row j = key, col i = query; keep iff j <= i), one copy per head
    tri = np.triu(np.ones((128, 128), dtype=np.float32))
    maskc = np.concatenate([tri, tri], axis=1).astype(bf16)
    sel = np.zeros((2, 128), dtype=np.float32)
    sel[0, 0:64] = 1.0
    sel[1, 64:128] = 1.0
    sel = sel.astype(bf16)
    in_maps = []
    for core in range(NCORES):
        b, g = core // 4, core % 4
        hs = g * HPC
        q_cols = w_attn[:, hs * HD:(hs + HPC) * HD]
        k_cols = w_attn[:, C + hs * HD: C + (hs + HPC) * HD]
        v_cols = w_attn[:, 2 * C + hs * HD: 2 * C + (hs + HPC) * HD]
        wqk = np.concatenate(
            [q_cols[:, 0:128], k_cols[:, 0:128], q_cols[:, 128:256], k_cols[:, 128:256]], axis=1)
        in_maps.append({
            "xT": np.ascontiguousarray(x[b].T).astype(bf16),
            "wqk": np.ascontiguousarray(wqk).astype(bf16),
            "wv": np.ascontiguousarray(v_cols).astype(bf16),
            "wp": np.ascontiguousarray(w_proj[hs * HD:(hs + HPC) * HD, :]).astype(bf16),
            "maskc": maskc,
            "sel": sel,
        })
    return in_maps


def run_cores(x, w_attn, w_proj, trace=False):
    nc = _get_nc()
    in_maps = _make_in_maps(x, w_attn, w_proj)
    res = run_bass_kernel_spmd(nc, in_maps, core_ids=list(range(NCORES)), trace=trace)
    out = np.zeros((B, T, C), dtype=np.float32)
    for core in range(NCORES):
        out[core // 4] += np.asarray(res.results[core]["out"], dtype=np.float32)
    return out, res


def kernel(x, w_attn, w_proj):
    out, _ = run_cores(x, w_attn, w_proj, trace=False)
    return out


# revision 26
# speedup vs baseline: 1.0089x; 1.0089x over previous
"""Causal self-attention on 8 Trainium2 NeuronCores.

Sharding: core = (batch b in {0,1}) x (head-group g in {0..3}), 4 heads per
core. Each core computes qkv for its heads from x[b], runs causal attention,
and multiplies by its 256 rows of w_proj, producing a partial [T, C] output.
Host sums the 4 partials per batch.

Layout: everything is computed "transposed" so no on-chip transposes are
needed. The host feeds x[b].T; q^T/k^T come out of the qkv matmul with
head-dim on partitions (exactly the S^T = K Q^T operand layout); softmax is
done on S^T (keys on partitions, queries on free) with the denominator
obtained by appending a ones-column to V in the A@V matmul; the A@V output
Y^T is exactly the lhsT layout the final projection needs.

v5 structure (v4 178us, v3 171.4us, baseline 222.9us):
- Input DMA prioritized: only the bytes pass0 needs (xT + the q0/k0
  halves of wqk, 4.5MB) go first; wv/mask/q1k1/wp ride behind.  The
  whole early phase is HBM-bandwidth-gated, so this directly pulls the
  first exp earlier.
- PE warmup burst (~3.6us) so HAM is at 8/8 when pass0 starts; pass0
  keeps ~100% PE duty against the DMA stream (8 matmuls per c-tile).
- Pair-major attention; stage-B tiles 0-3 inline in qc0; pass1 and
  B4-15 drip as PE filler units; per-pair normalization.
- Tail: all reserve projection groups are emitted BEFORE the
  normalization chain (the PE queue is in-order - fillers emitted after
  a blocked instruction never fill anything), sums DMAs issued before
  the ytr shifts, and the last groups alternate PSUM pools and
  eviction engines to pipeline 4 deep.
"""

import numpy as np
import ml_dtypes

import concourse.bass as bass
import concourse.bacc as bacc
import concourse.tile as tile
from concourse import mybir
from concourse.bass_utils import run_bass_kernel_spmd

F32 = mybir.dt.float32
BF16 = mybir.dt.bfloat16
EXP = mybir.ActivationFunctionType.Exp

B, T, C, H, HD = 2, 2048, 1024, 16, 64
NCORES = 8
HPC = 4      # heads per core
NPAIR = 2    # head pairs per core
NCT = C // 128   # 8 c-tiles
NTT = T // 128   # 16 t-tiles
NQC = T // 512   # 4 query chunks
SCALE = 1.0 / np.sqrt(HD)
NEG = -1.0e30


def build_kernel():
    nc = bacc.Bacc("TRN2", target_bir_lowering=False, debug=False, num_devices=NCORES)

    xT = nc.dram_tensor("xT", [C, T], BF16, kind="ExternalInput")
    wqk = nc.dram_tensor("wqk", [C, 512], BF16, kind="ExternalInput")
    wv = nc.dram_tensor("wv", [C, 256], BF16, kind="ExternalInput")
    wp = nc.dram_tensor("wp", [256, C], BF16, kind="ExternalInput")
    maskc = nc.dram_tensor("maskc", [128, 256], BF16, kind="ExternalInput")
    sel = nc.dram_tensor("sel", [2, 128], BF16, kind="ExternalInput")
    out = nc.dram_tensor("out", [T, C], BF16, kind="ExternalOutput")

    with tile.TileContext(nc) as tc:
        _body(tc, xT, wqk, wv, wp, maskc, sel, out)

    nc.compile()
    return nc


def _body(tc, xT, wqk, wv, wp, maskc, sel, out):
    nc = tc.nc
    from contextlib import ExitStack

    with ExitStack() as ctx:
        sb = lambda name: ctx.enter_context(tc.tile_pool(name=name, bufs=1))
        qkT_sb = sb("qkT").tile([128, 4 * T], BF16)       # bands q0,k0,q1,k1
        v65_sb = sb("v65").tile([128, NTT * 260], BF16)   # per k-tile: 4x(64 v + 1 ones)
        yt_sb = sb("yt").tile([128, NPAIR * T], BF16)     # pair p: rows 0-63 head 2p, 64-127 head 2p+1
        wp_sb = sb("wp").tile([128, 2 * C], BF16)
        maskc_sb = sb("maskc").tile([128, 256], BF16)
        sel_sb = sb("sel").tile([2, 128], BF16)
        wu_sb = sb("wu").tile([128, 128], BF16)

        es_pool = ctx.enter_context(tc.tile_pool(name="es", bufs=5))
        sums_pool = ctx.enter_context(tc.tile_pool(name="sums", bufs=2))
        rc_pool = ctx.enter_context(tc.tile_pool(name="rc", bufs=2))
        avst_pool = ctx.enter_context(tc.tile_pool(name="avst", bufs=2))
        ytr_pool = ctx.enter_context(tc.tile_pool(name="ytr", bufs=2))
        ost_pool = ctx.enter_context(tc.tile_pool(name="ost", bufs=3))

        # PSUM: psS tag = 2 slots x [128,1024] (4 banks), av 2 banks, misc 2
        ps = ctx.enter_context(tc.tile_pool(name="ps", bufs=2, space="PSUM"))
        av_pool = ctx.enter_context(tc.tile_pool(name="av", bufs=2, space="PSUM"))
        psS_pool = ctx.enter_context(tc.tile_pool(name="psS", bufs=2, space="PSUM"))

        xw_pool = ctx.enter_context(tc.tile_pool(name="xw", bufs=1))
        xT_sb = xw_pool.tile([128, NCT * T], BF16, name="xT_sb")
        wqk_sb = xw_pool.tile([128, NCT * 512], BF16, name="wqk_sb")
        wv_sb = xw_pool.tile([128, NCT * 256], BF16, name="wv_sb")

        # loads: ONLY what pass0 needs first (bands q0,k0 = first 256 cols of
        # each wqk c-tile + xT) - the early phase is HBM-bandwidth-gated.
        for c in range(NCT):
            nc.sync.dma_start(wqk_sb[:, c * 512:c * 512 + 256], wqk[c * 128:(c + 1) * 128, 0:256])
            nc.sync.dma_start(xT_sb[:, c * T:(c + 1) * T], xT[c * 128:(c + 1) * 128, :])
        nc.sync.dma_start(
            wv_sb[:].rearrange("p (c j) -> p c j", c=NCT),
            wv.ap().rearrange("(c p) j -> p c j", c=NCT))
        nc.sync.dma_start(maskc_sb[:], maskc[:])
        nc.sync.dma_start(sel_sb[:], sel[:])
        nc.sync.dma_start(  # bands q1,k1 (cols 256:512 of each c-tile)
            wqk_sb[:].rearrange("p (c j) -> p c j", c=NCT)[:, :, 256:512],
            wqk.ap().rearrange("(c p) j -> p c j", c=NCT)[:, :, 256:512])
        nc.sync.dma_start(
            wp_sb[:].rearrange("p (q c) -> p q c", q=2),
            wp.ap().rearrange("(q p) c -> p q c", q=2))
        v65_4d = v65_sb[:].rearrange("p (t h d) -> p t h d", t=NTT, h=HPC, d=65)
        nc.vector.memset(v65_4d[:, :, :, 64:65], 1.0)
        nc.vector.memset(wu_sb[:], 0.0)

        # ---- PE warmup: ~3.6us of back-to-back dummy matmuls while the
        # first input DMAs land, so HAM un-throttles the PE clock.
        wu_ps = psS_pool.tile([128, 1024], F32, tag="psS", name="warmup")
        for i in range(34):
            nc.tensor.matmul(wu_ps[:, 0:128], wu_sb[:], wu_sb[:], start=True, stop=True)

        # ---- stage A pass0: bands q0,k0 (all four t-chunks), c-outer so
        # each xT c-tile feeds its 8 matmuls the moment the DMA lands;
        # PE duty stays ~100% against the DMA stream, keeping HAM warm.
        accs = {}
        for b in (0, 1):
            acc01 = psS_pool.tile([128, 1024], F32, tag="psS", name=f"accA_{b}_01")
            acc2 = av_pool.tile([128, 512], F32, tag="av", name=f"accA_{b}_2")
            acc3 = ps.tile([128, 512], F32, tag="ps", name=f"accA_{b}_3")
            accs[b] = [acc01, acc2, acc3]
        for c in range(NCT):
            for b in (0, 1):
                lhs = wqk_sb[:, c * 512 + b * 128: c * 512 + (b + 1) * 128]
                acc01, acc2, acc3 = accs[b]
                dsts = [acc01[:, 0:512], acc01[:, 512:1024], acc2[:], acc3[:]]
                for t4 in range(4):
                    nc.tensor.matmul(
                        dsts[t4], lhs,
                        xT_sb[:, c * T + t4 * 512: c * T + (t4 + 1) * 512],
                        start=(c == 0), stop=(c == NCT - 1))
        for b in (0, 1):
            acc01, acc2, acc3 = accs[b]
            nc.vector.tensor_copy(qkT_sb[:, b * T: b * T + 1024], acc01[:])
            nc.vector.tensor_copy(qkT_sb[:, b * T + 1024: b * T + 1536], acc2[:])
            nc.vector.tensor_copy(qkT_sb[:, b * T + 1536: b * T + 2048], acc3[:])

        # ---- stage A pass1 (bands q1,k1): single-bank chunks over the 8
        # c-tiles, emitted as two 4-matmul pending units (PE filler).
        def make_A_chunk(b, t4):
            state = {}

            def half1():
                state["acc"] = ps.tile([128, 512], F32, tag="ps", name=f"accA_{b}_{t4}")
                for c in range(4):
                    nc.tensor.matmul(state["acc"][:],
                                     wqk_sb[:, c * 512 + b * 128: c * 512 + (b + 1) * 128],
                                     xT_sb[:, c * T + t4 * 512: c * T + (t4 + 1) * 512],
                                     start=(c == 0), stop=False)

            def half2():
                for c in range(4, NCT):
                    nc.tensor.matmul(state["acc"][:],
                                     wqk_sb[:, c * 512 + b * 128: c * 512 + (b + 1) * 128],
                                     xT_sb[:, c * T + t4 * 512: c * T + (t4 + 1) * 512],
                                     start=False, stop=(c == NCT - 1))
                nc.vector.tensor_copy(qkT_sb[:, b * T + t4 * 512: b * T + (t4 + 1) * 512],
                                      state["acc"][:])

            return [half1, half2]

        # ---- stage B: v natural [t, j]
        def emit_B(t):
            psv = ps.tile([128, 512], F32, tag="ps", name=f"psv_{t}")
            for c in range(NCT):
                lhs = xT_sb[:, c * T + t * 128: c * T + (t + 1) * 128]
                nc.tensor.matmul(psv[:, 0:256], lhs, wv_sb[:, c * 256:(c + 1) * 256],
                                 start=(c == 0), stop=(c == NCT - 1))
            dst = v65_sb[:, t * 260:(t + 1) * 260].rearrange("p (h d) -> p h d", h=HPC, d=65)
            src_ = psv[:, 0:256].rearrange("p (h d) -> p h d", h=HPC, d=64)
            nc.vector.tensor_copy(dst[:, :, 0:64], src_)

        # ---- stage D: projection (needs both pairs' normalized Y^T)
        def emit_proj_group(t, n, ev="v", pool=None):
            pool = pool or ps
            pso = pool.tile([128, 512] if pool is ps else [128, 1024], F32,
                            tag="ps" if pool is ps else "psS", name=f"pso_{t}_{n}")
            for p in range(NPAIR):
                lhsT = yt_sb[:, p * T + t * 128: p * T + (t + 1) * 128]
                rhs = wp_sb[:, p * C + n * 512: p * C + (n + 1) * 512]
                nc.tensor.matmul(pso[:, 0:512], lhsT, rhs, start=(p == 0), stop=(p == NPAIR - 1))
            ost = ost_pool.tile([128, 512], BF16, tag="ost", name=f"ost_{t}_{n}")
            if ev == "s":
                nc.scalar.copy(ost[:], pso[:, 0:512])
            else:
                nc.vector.tensor_copy(ost[:], pso[:, 0:512])
            nc.sync.dma_start(out[t * 128:(t + 1) * 128, n * 512:(n + 1) * 512], ost[:])

        pending = []
        for t4 in (0, 1):
            pending += make_A_chunk(2, t4) + make_A_chunk(3, t4)
            pending += [lambda t=t: emit_B(t) for t in range(4 + 6 * t4, 10 + 6 * t4)]
        # pass1's t2/t3 chunks are only needed by pair1's qc2/qc3 - hold
        # them back to fill pair1's drip-dry early chunks (otherwise the
        # pair transition idles the PE past the HAM window).
        late = []
        for t4 in (2, 3):
            late += make_A_chunk(2, t4) + make_A_chunk(3, t4)
        reserve = []  # drained right before the tail normalization chain

        def run_pair(p):
            qb, kb = 2 * p, 2 * p + 1
            for qc in range(NQC):
                nkt = 4 * qc + 4
                av = [av_pool.tile([128, 512], F32, tag="av", name=f"av_{p}_{qc}_{i}") for i in range(2)]

                def emit_S(kt, qc=qc):
                    psb = psS_pool.tile([128, 1024], F32, tag="psS", name=f"psS_{p}_{qc}_{kt}")
                    slo = max(kt - 4 * qc, 0) * 128
                    for h in range(2):
                        base = 64 * h
                        lhsT = qkT_sb[base:base + 64, kb * T + kt * 128: kb * T + (kt + 1) * 128]
                        rhs = qkT_sb[base:base + 64, qb * T + qc * 512 + slo: qb * T + (qc + 1) * 512]
                        nc.tensor.matmul(psb[:, h * 512 + slo:(h + 1) * 512], lhsT, rhs,
                                         start=True, stop=True, tile_position=(base, 0))
                    return psb

                pipe = [emit_S(0)]
                if nkt > 1:
                    pipe.append(emit_S(1))
                for kt in range(nkt):
                    cur = pipe.pop(0)
                    if kt + 2 < nkt:
                        pipe.append(emit_S(kt + 2))
                    d = kt - 4 * qc
                    lo = max(d, 0) * 128  # first valid query column of this k-tile
                    psb2 = cur[:].rearrange("p (h q) -> p h q", h=2, q=512)
                    es = es_pool.tile([128, 1024], BF16, tag="es", name=f"es_{p}_{qc}_{kt}")
                    es2 = es[:].rearrange("p (h q) -> p h q", h=2, q=512)
                    nc.scalar.activation(es2[:, :, lo:], psb2[:, :, lo:], EXP, scale=SCALE)
                    if d >= 0:
                        # causal mask: zero the upper triangle of the diag
                        # block AFTER exp (bf16 multiply, off the S->exp
                        # critical path; the ones-column denominators sum
                        # es after this, so they stay exact)
                        nc.vector.tensor_mul(
                            es2[:, :, lo:lo + 128], es2[:, :, lo:lo + 128],
                            maskc_sb[:].rearrange("p (h q) -> p h q", h=2, q=128))
                    if p == 0 and qc == 0:
                        emit_B(kt)  # v tiles 0-3, needed by this chunk's AV
                    for h in range(2):
                        hh = 2 * p + h
                        lhsT_v = v65_sb[:, kt * 260 + hh * 65: kt * 260 + (hh + 1) * 65]
                        nc.tensor.matmul(av[h][0:65, lo:], lhsT_v, es[:, h * 512 + lo:(h + 1) * 512],
                                         start=(kt == 0), stop=(kt == nkt - 1))
                    popped = 0
                    lim = 2 if len(pending) > 8 else 1
                    while pending and popped < lim:
                        pending.pop(0)()
                        popped += 1
                # evict Y^T + sums; sums DMAs go FIRST (they gate the
                # reciprocal chain), the ytr partition-shift DMAs after.
                ytr = ytr_pool.tile([128, 512], F32, tag="ytr", name=f"ytr_{p}_{qc}")
                sums2 = sums_pool.tile([2, 512], F32, tag="sums", name=f"sums_{p}_{qc}")
                sts = []
                tail_qc = p == 1 and qc == NQC - 1
                for h in range(2):
                    st = avst_pool.tile([65, 512], F32, tag="avst", name=f"avst_{p}_{qc}_{h}")
                    if tail_qc:
                        nc.scalar.copy(st[:], av[h][0:65, :])  # ACT is idle post-exp
                    else:
                        nc.vector.tensor_copy(st[:], av[h][0:65, :])
                    nc.sync.dma_start(sums2[h:h + 1, :], st[64:65, :])
                    sts.append(st)
                for h in range(2):
                    nc.sync.dma_start(ytr[64 * h:64 * (h + 1), :], sts[h][0:64, :])

                def norm_pair(qc=qc, ytr=ytr, sums2=sums2):
                    rcf = rc_pool.tile([2, 512], F32, tag="rcf", name=f"rcf_{p}_{qc}")
                    rc2 = rc_pool.tile([2, 512], BF16, tag="rc", name=f"rc_{p}_{qc}")
                    nc.vector.reciprocal_approx_fast(rcf[:], sums2[:])
                    nc.vector.tensor_copy(rc2[:], rcf[:])
                    psR = ps.tile([128, 512], F32, tag="ps", name=f"psR_{p}_{qc}")
                    nc.tensor.matmul(psR[:], sel_sb[:], rc2[:], start=True, stop=True)
                    nc.vector.tensor_mul(yt_sb[:, p * T + qc * 512: p * T + (qc + 1) * 512],
                                         ytr[:], psR[:])

                if p == 1 and qc == NQC - 1:
                    # tail: ALL reserve groups first (in-order PE queue -
                    # they fill the normalization chain's latency), then the
                    # last 8 groups 4-deep across two PSUM pools with
                    # alternating eviction engines.
                    while reserve:
                        reserve.pop(0)()
                    norm_pair()
                    for i, (t, n) in enumerate([(t, n) for t in range(12, 16) for n in range(2)]):
                        emit_proj_group(t, n, ev="sv"[i % 2], pool=(ps, psS_pool)[i % 2])
                else:
                    pending.append(norm_pair)
                if p == 1 and qc < NQC - 1:
                    items = [lambda t=t, n=n: emit_proj_group(t, n)
                             for t in range(4 * qc, 4 * qc + 4) for n in range(2)]
                    if qc == NQC - 2:
                        pending.extend(items[:3])
                        reserve.extend(items[3:])
                    elif qc == NQC - 3:
                        pending.extend(items[:5])
                        reserve.extend(items[5:])
                    else:
                        pending.extend(items)

        run_pair(0)
        pending[0:0] = late
        run_pair(1)
        for fn in pending:
            fn()


_NC_CACHE = None


def _get_nc():
    global _NC_CACHE
    if _NC_CACHE is None:
        _NC_CACHE = build_kernel()
    return _NC_CACHE


def _make_in_maps(x, w_attn, w_proj):
    bf16 = ml_dtypes.bfloat16
    x = np.asarray(x, dtype=np.float32)
    w_attn = np.asarray(w_attn, dtype=np.float32)
    w_proj = np.asarray(w_proj, dtype=np.float32)
    # maskc: [128, 2x128] 0/1 keep-mask for the diagonal blocks
    # (row j = key, col i = query; keep iff j <= i), one copy per head
    tri = np.triu(np.ones((128, 128), dtype=np.float32))
    maskc = np.concatenate([tri, tri], axis=1).astype(bf16)
    sel = np.zeros((2, 128), dtype=np.float32)
    sel[0, 0:64] = 1.0
    sel[1, 64:128] = 1.0
    sel = sel.astype(bf16)
    in_maps = []
    for core in range(NCORES):
        b, g = core // 4, core % 4
        hs = g * HPC
        q_cols = w_attn[:, hs * HD:(hs + HPC) * HD]
        k_cols = w_attn[:, C + hs * HD: C + (hs + HPC) * HD]
        v_cols = w_attn[:, 2 * C + hs * HD: 2 * C + (hs + HPC) * HD]
        wqk = np.concatenate(
            [q_cols[:, 0:128], k_cols[:, 0:128], q_cols[:, 128:256], k_cols[:, 128:256]], axis=1)
        in_maps.append({
            "xT": np.ascontiguousarray(x[b].T).astype(bf16),
            "wqk": np.ascontiguousarray(wqk).astype(bf16),
            "wv": np.ascontiguousarray(v_cols).astype(bf16),
            "wp": np.ascontiguousarray(w_proj[hs * HD:(hs + HPC) * HD, :]).astype(bf16),
            "maskc": maskc,
            "sel": sel,
        })
    return in_maps


def run_cores(x, w_attn, w_proj, trace=False):
    nc = _get_nc()
    in_maps = _make_in_maps(x, w_attn, w_proj)
    res = run_bass_kernel_spmd(nc, in_maps, core_ids=list(range(NCORES)), trace=trace)
    out = np.zeros((B, T, C), dtype=np.float32)
    for core in range(NCORES):
        out[core // 4] += np.asarray(res.results[core]["out"], dtype=np.float32)
    return out, res


def kernel(x, w_attn, w_proj):
    out, _ = run_cores(x, w_attn, w_proj, trace=False)
    return out


# revision 27
# speedup vs baseline: 1.0109x; 1.0020x over previous
"""Causal self-attention on 8 Trainium2 NeuronCores.

Sharding: core = (batch b in {0,1}) x (head-group g in {0..3}), 4 heads per
core. Each core computes qkv for its heads from x[b], runs causal attention,
and multiplies by its 256 rows of w_proj, producing a partial [T, C] output.
Host sums the 4 partials per batch.

Layout: everything is computed "transposed" so no on-chip transposes are
needed. The host feeds x[b].T; q^T/k^T come out of the qkv matmul with
head-dim on partitions (exactly the S^T = K Q^T operand layout); softmax is
done on S^T (keys on partitions, queries on free) with the denominator
obtained by appending a ones-column to V in the A@V matmul; the A@V output
Y^T is exactly the lhsT layout the final projection needs.

v5 structure (v4 178us, v3 171.4us, baseline 222.9us):
- Input DMA prioritized: only the bytes pass0 needs (xT + the q0/k0
  halves of wqk, 4.5MB) go first; wv/mask/q1k1/wp ride behind.  The
  whole early phase is HBM-bandwidth-gated, so this directly pulls the
  first exp earlier.
- PE warmup burst (~3.6us) so HAM is at 8/8 when pass0 starts; pass0
  keeps ~100% PE duty against the DMA stream (8 matmuls per c-tile).
- Pair-major attention; stage-B tiles 0-3 inline in qc0; pass1 and
  B4-15 drip as PE filler units; per-pair normalization.
- Tail: all reserve projection groups are emitted BEFORE the
  normalization chain (the PE queue is in-order - fillers emitted after
  a blocked instruction never fill anything), sums DMAs issued before
  the ytr shifts, and the last groups alternate PSUM pools and
  eviction engines to pipeline 4 deep.
"""

import numpy as np
import ml_dtypes

import concourse.bass as bass
import concourse.bacc as bacc
import concourse.tile as tile
from concourse import mybir
from concourse.bass_utils import run_bass_kernel_spmd

F32 = mybir.dt.float32
BF16 = mybir.dt.bfloat16
EXP = mybir.ActivationFunctionType.Exp

B, T, C, H, HD = 2, 2048, 1024, 16, 64
NCORES = 8
HPC = 4      # heads per core
NPAIR = 2    # head pairs per core
NCT = C // 128   # 8 c-tiles
NTT = T // 128   # 16 t-tiles
NQC = T // 512   # 4 query chunks
SCALE = 1.0 / np.sqrt(HD)
NEG = -1.0e30


def build_kernel():
    nc = bacc.Bacc("TRN2", target_bir_lowering=False, debug=False, num_devices=NCORES)

    xT = nc.dram_tensor("xT", [C, T], BF16, kind="ExternalInput")
    wqk = nc.dram_tensor("wqk", [C, 512], BF16, kind="ExternalInput")
    wv = nc.dram_tensor("wv", [C, 256], BF16, kind="ExternalInput")
    wp = nc.dram_tensor("wp", [256, C], BF16, kind="ExternalInput")
    maskc = nc.dram_tensor("maskc", [128, 256], BF16, kind="ExternalInput")
    sel = nc.dram_tensor("sel", [2, 128], BF16, kind="ExternalInput")
    out = nc.dram_tensor("out", [T, C], BF16, kind="ExternalOutput")

    with tile.TileContext(nc) as tc:
        _body(tc, xT, wqk, wv, wp, maskc, sel, out)

    nc.compile()
    return nc


def _body(tc, xT, wqk, wv, wp, maskc, sel, out):
    nc = tc.nc
    from contextlib import ExitStack

    with ExitStack() as ctx:
        sb = lambda name: ctx.enter_context(tc.tile_pool(name=name, bufs=1))
        qkT_sb = sb("qkT").tile([128, 4 * T], BF16)       # bands q0,k0,q1,k1
        v65_sb = sb("v65").tile([128, NTT * 260], BF16)   # per k-tile: 4x(64 v + 1 ones)
        yt_sb = sb("yt").tile([128, NPAIR * T], BF16)     # pair p: rows 0-63 head 2p, 64-127 head 2p+1
        wp_sb = sb("wp").tile([128, 2 * C], BF16)
        maskc_sb = sb("maskc").tile([128, 256], BF16)
        sel_sb = sb("sel").tile([2, 128], BF16)
        wu_sb = sb("wu").tile([128, 128], BF16)

        es_pool = ctx.enter_context(tc.tile_pool(name="es", bufs=4))
        sums_pool = ctx.enter_context(tc.tile_pool(name="sums", bufs=2))
        rc_pool = ctx.enter_context(tc.tile_pool(name="rc", bufs=2))
        avst_pool = ctx.enter_context(tc.tile_pool(name="avst", bufs=2))
        ytr_pool = ctx.enter_context(tc.tile_pool(name="ytr", bufs=2))
        ost_pool = ctx.enter_context(tc.tile_pool(name="ost", bufs=3))

        # PSUM: psS tag = 2 slots x [128,1024] (4 banks), av 2 banks, misc 2
        ps = ctx.enter_context(tc.tile_pool(name="ps", bufs=2, space="PSUM"))
        av_pool = ctx.enter_context(tc.tile_pool(name="av", bufs=2, space="PSUM"))
        psS_pool = ctx.enter_context(tc.tile_pool(name="psS", bufs=2, space="PSUM"))

        xw_pool = ctx.enter_context(tc.tile_pool(name="xw", bufs=1))
        xT_sb = xw_pool.tile([128, NCT * T], BF16, name="xT_sb")
        wqk_sb = xw_pool.tile([128, NCT * 512], BF16, name="wqk_sb")
        wv_sb = xw_pool.tile([128, NCT * 256], BF16, name="wv_sb")

        # loads: ONLY what pass0 needs first (bands q0,k0 = first 256 cols of
        # each wqk c-tile + xT) - the early phase is HBM-bandwidth-gated.
        for c in range(NCT):
            nc.sync.dma_start(wqk_sb[:, c * 512:c * 512 + 256], wqk[c * 128:(c + 1) * 128, 0:256])
            nc.sync.dma_start(xT_sb[:, c * T:(c + 1) * T], xT[c * 128:(c + 1) * 128, :])
        nc.sync.dma_start(
            wv_sb[:].rearrange("p (c j) -> p c j", c=NCT),
            wv.ap().rearrange("(c p) j -> p c j", c=NCT))
        nc.sync.dma_start(maskc_sb[:], maskc[:])
        nc.sync.dma_start(sel_sb[:], sel[:])
        nc.sync.dma_start(  # bands q1,k1 (cols 256:512 of each c-tile)
            wqk_sb[:].rearrange("p (c j) -> p c j", c=NCT)[:, :, 256:512],
            wqk.ap().rearrange("(c p) j -> p c j", c=NCT)[:, :, 256:512])
        nc.sync.dma_start(
            wp_sb[:].rearrange("p (q c) -> p q c", q=2),
            wp.ap().rearrange("(q p) c -> p q c", q=2))
        v65_4d = v65_sb[:].rearrange("p (t h d) -> p t h d", t=NTT, h=HPC, d=65)
        nc.vector.memset(v65_4d[:, :, :, 64:65], 1.0)
        nc.vector.memset(wu_sb[:], 0.0)

        # ---- PE warmup: ~3.6us of back-to-back dummy matmuls while the
        # first input DMAs land, so HAM un-throttles the PE clock.
        wu_ps = psS_pool.tile([128, 1024], F32, tag="psS", name="warmup")
        for i in range(34):
            nc.tensor.matmul(wu_ps[:, 0:128], wu_sb[:], wu_sb[:], start=True, stop=True)

        # ---- stage A pass0: bands q0,k0 (all four t-chunks), c-outer so
        # each xT c-tile feeds its 8 matmuls the moment the DMA lands;
        # PE duty stays ~100% against the DMA stream, keeping HAM warm.
        accs = {}
        for b in (0, 1):
            acc01 = psS_pool.tile([128, 1024], F32, tag="psS", name=f"accA_{b}_01")
            acc2 = av_pool.tile([128, 512], F32, tag="av", name=f"accA_{b}_2")
            acc3 = ps.tile([128, 512], F32, tag="ps", name=f"accA_{b}_3")
            accs[b] = [acc01, acc2, acc3]
        for c in range(NCT):
            for b in (0, 1):
                lhs = wqk_sb[:, c * 512 + b * 128: c * 512 + (b + 1) * 128]
                acc01, acc2, acc3 = accs[b]
                dsts = [acc01[:, 0:512], acc01[:, 512:1024], acc2[:], acc3[:]]
                for t4 in range(4):
                    nc.tensor.matmul(
                        dsts[t4], lhs,
                        xT_sb[:, c * T + t4 * 512: c * T + (t4 + 1) * 512],
                        start=(c == 0), stop=(c == NCT - 1))
        for b in (0, 1):
            acc01, acc2, acc3 = accs[b]
            nc.vector.tensor_copy(qkT_sb[:, b * T: b * T + 1024], acc01[:])
            nc.vector.tensor_copy(qkT_sb[:, b * T + 1024: b * T + 1536], acc2[:])
            nc.vector.tensor_copy(qkT_sb[:, b * T + 1536: b * T + 2048], acc3[:])

        # ---- stage A pass1 (bands q1,k1): single-bank chunks over the 8
        # c-tiles, emitted as two 4-matmul pending units (PE filler).
        def make_A_chunk(b, t4):
            state = {}

            def half1():
                state["acc"] = ps.tile([128, 512], F32, tag="ps", name=f"accA_{b}_{t4}")
                for c in range(4):
                    nc.tensor.matmul(state["acc"][:],
                                     wqk_sb[:, c * 512 + b * 128: c * 512 + (b + 1) * 128],
                                     xT_sb[:, c * T + t4 * 512: c * T + (t4 + 1) * 512],
                                     start=(c == 0), stop=False)

            def half2():
                for c in range(4, NCT):
                    nc.tensor.matmul(state["acc"][:],
                                     wqk_sb[:, c * 512 + b * 128: c * 512 + (b + 1) * 128],
                                     xT_sb[:, c * T + t4 * 512: c * T + (t4 + 1) * 512],
                                     start=False, stop=(c == NCT - 1))
                nc.vector.tensor_copy(qkT_sb[:, b * T + t4 * 512: b * T + (t4 + 1) * 512],
                                      state["acc"][:])

            return [half1, half2]

        # ---- stage B: v natural [t, j]
        def emit_B(t):
            psv = ps.tile([128, 512], F32, tag="ps", name=f"psv_{t}")
            for c in range(NCT):
                lhs = xT_sb[:, c * T + t * 128: c * T + (t + 1) * 128]
                nc.tensor.matmul(psv[:, 0:256], lhs, wv_sb[:, c * 256:(c + 1) * 256],
                                 start=(c == 0), stop=(c == NCT - 1))
            dst = v65_sb[:, t * 260:(t + 1) * 260].rearrange("p (h d) -> p h d", h=HPC, d=65)
            src_ = psv[:, 0:256].rearrange("p (h d) -> p h d", h=HPC, d=64)
            nc.vector.tensor_copy(dst[:, :, 0:64], src_)

        # ---- stage D: projection (needs both pairs' normalized Y^T)
        def emit_proj_group(t, n, ev="v", pool=None):
            pool = pool or ps
            pso = pool.tile([128, 512] if pool is ps else [128, 1024], F32,
                            tag="ps" if pool is ps else "psS", name=f"pso_{t}_{n}")
            for p in range(NPAIR):
                lhsT = yt_sb[:, p * T + t * 128: p * T + (t + 1) * 128]
                rhs = wp_sb[:, p * C + n * 512: p * C + (n + 1) * 512]
                nc.tensor.matmul(pso[:, 0:512], lhsT, rhs, start=(p == 0), stop=(p == NPAIR - 1))
            ost = ost_pool.tile([128, 512], BF16, tag="ost", name=f"ost_{t}_{n}")
            if ev == "s":
                nc.scalar.copy(ost[:], pso[:, 0:512])
            else:
                nc.vector.tensor_copy(ost[:], pso[:, 0:512])
            nc.sync.dma_start(out[t * 128:(t + 1) * 128, n * 512:(n + 1) * 512], ost[:])

        pending = []
        for t4 in (0, 1):
            pending += make_A_chunk(2, t4) + make_A_chunk(3, t4)
            pending += [lambda t=t: emit_B(t) for t in range(4 + 6 * t4, 10 + 6 * t4)]
        # pass1's t2/t3 chunks are only needed by pair1's qc2/qc3 - hold
        # them back to fill pair1's drip-dry early chunks (otherwise the
        # pair transition idles the PE past the HAM window).
        late = []
        for t4 in (2, 3):
            late += make_A_chunk(2, t4) + make_A_chunk(3, t4)
        reserve = []  # drained right before the tail normalization chain

        def run_pair(p):
            qb, kb = 2 * p, 2 * p + 1
            for qc in range(NQC):
                nkt = 4 * qc + 4
                av = [av_pool.tile([128, 512], F32, tag="av", name=f"av_{p}_{qc}_{i}") for i in range(2)]

                def emit_S(kt, qc=qc):
                    psb = psS_pool.tile([128, 1024], F32, tag="psS", name=f"psS_{p}_{qc}_{kt}")
                    slo = max(kt - 4 * qc, 0) * 128
                    for h in range(2):
                        base = 64 * h
                        lhsT = qkT_sb[base:base + 64, kb * T + kt * 128: kb * T + (kt + 1) * 128]
                        rhs = qkT_sb[base:base + 64, qb * T + qc * 512 + slo: qb * T + (qc + 1) * 512]
                        nc.tensor.matmul(psb[:, h * 512 + slo:(h + 1) * 512], lhsT, rhs,
                                         start=True, stop=True, tile_position=(base, 0))
                    return psb

                pipe = [emit_S(0)]
                if nkt > 1:
                    pipe.append(emit_S(1))
                for kt in range(nkt):
                    cur = pipe.pop(0)
                    if kt + 2 < nkt:
                        pipe.append(emit_S(kt + 2))
                    d = kt - 4 * qc
                    lo = max(d, 0) * 128  # first valid query column of this k-tile
                    psb2 = cur[:].rearrange("p (h q) -> p h q", h=2, q=512)
                    es = es_pool.tile([128, 1024], BF16, tag="es", name=f"es_{p}_{qc}_{kt}")
                    es2 = es[:].rearrange("p (h q) -> p h q", h=2, q=512)
                    nc.scalar.activation(es2[:, :, lo:], psb2[:, :, lo:], EXP, scale=SCALE)
                    if d >= 0:
                        # causal mask: zero the upper triangle of the diag
                        # block AFTER exp (bf16 multiply, off the S->exp
                        # critical path; the ones-column denominators sum
                        # es after this, so they stay exact)
                        nc.vector.tensor_mul(
                            es2[:, :, lo:lo + 128], es2[:, :, lo:lo + 128],
                            maskc_sb[:].rearrange("p (h q) -> p h q", h=2, q=128))
                    if p == 0 and qc == 0:
                        emit_B(kt)  # v tiles 0-3, needed by this chunk's AV
                    for h in range(2):
                        hh = 2 * p + h
                        lhsT_v = v65_sb[:, kt * 260 + hh * 65: kt * 260 + (hh + 1) * 65]
                        nc.tensor.matmul(av[h][0:65, lo:], lhsT_v, es[:, h * 512 + lo:(h + 1) * 512],
                                         start=(kt == 0), stop=(kt == nkt - 1))
                    popped = 0
                    lim = 2 if len(pending) > 8 else 1
                    while pending and popped < lim:
                        pending.pop(0)()
                        popped += 1
                # evict Y^T + sums; sums DMAs go FIRST (they gate the
                # reciprocal chain), the ytr partition-shift DMAs after.
                ytr = ytr_pool.tile([128, 512], F32, tag="ytr", name=f"ytr_{p}_{qc}")
                sums2 = sums_pool.tile([2, 512], F32, tag="sums", name=f"sums_{p}_{qc}")
                sts = []
                tail_qc = p == 1 and qc == NQC - 1
                for h in range(2):
                    st = avst_pool.tile([65, 512], F32, tag="avst", name=f"avst_{p}_{qc}_{h}")
                    if tail_qc:
                        nc.scalar.copy(st[:], av[h][0:65, :])  # ACT is idle post-exp
                    else:
                        nc.vector.tensor_copy(st[:], av[h][0:65, :])
                    nc.sync.dma_start(sums2[h:h + 1, :], st[64:65, :])
                    sts.append(st)
                for h in range(2):
                    nc.sync.dma_start(ytr[64 * h:64 * (h + 1), :], sts[h][0:64, :])

                def norm_pair(qc=qc, ytr=ytr, sums2=sums2):
                    rcf = rc_pool.tile([2, 512], F32, tag="rcf", name=f"rcf_{p}_{qc}")
                    rc2 = rc_pool.tile([2, 512], BF16, tag="rc", name=f"rc_{p}_{qc}")
                    nc.vector.reciprocal_approx_fast(rcf[:], sums2[:])
                    nc.vector.tensor_copy(rc2[:], rcf[:])
                    psR = ps.tile([128, 512], F32, tag="ps", name=f"psR_{p}_{qc}")
                    nc.tensor.matmul(psR[:], sel_sb[:], rc2[:], start=True, stop=True)
                    nc.vector.tensor_mul(yt_sb[:, p * T + qc * 512: p * T + (qc + 1) * 512],
                                         ytr[:], psR[:])

                if p == 1 and qc == NQC - 1:
                    # tail: ALL reserve groups first (in-order PE queue -
                    # they fill the normalization chain's latency), then the
                    # last 8 groups 4-deep across two PSUM pools with
                    # alternating eviction engines.
                    while reserve:
                        reserve.pop(0)()
                    norm_pair()
                    for i, (t, n) in enumerate([(t, n) for t in range(12, 16) for n in range(2)]):
                        emit_proj_group(t, n, ev="sv"[i % 2], pool=(ps, psS_pool)[i % 2])
                else:
                    pending.append(norm_pair)
                if p == 1 and qc < NQC - 1:
                    items = [lambda t=t, n=n: emit_proj_group(t, n)
                             for t in range(4 * qc, 4 * qc + 4) for n in range(2)]
                    if qc == NQC - 2:
                        pending.extend(items[:3])
                        reserve.extend(items[3:])
                    elif qc == NQC - 3:
                        pending.extend(items[:5])
                        reserve.extend(items[5:])
                    else:
                        pending.extend(items)

        run_pair(0)
        pending[0:0] = late
        run_pair(1)
        for fn in pending:
            fn()


_NC_CACHE = None


def _get_nc():
    global _NC_CACHE
    if _NC_CACHE is None:
        _NC_CACHE = build_kernel()
    return _NC_CACHE


def _make_in_maps(x, w_attn, w_proj):
    bf16 = ml_dtypes.bfloat16
    x = np.asarray(x, dtype=np.float32)
    w_attn = np.asarray(w_attn, dtype=np.float32)
    w_proj = np.asarray(w_proj, dtype=np.float32)
    # maskc: [128, 2x128] 0/1 keep-mask for the diagonal blocks
    # (row j = key, col i = query; keep iff j <= i), one copy per head
    tri = np.triu(np.ones((128, 128), dtype=np.float32))
    maskc = np.concatenate([tri, tri], axis=1).astype(bf16)
    sel = np.zeros((2, 128), dtype=np.float32)
    sel[0, 0:64] = 1.0
    sel[1, 64:128] = 1.0
    sel = sel.astype(bf16)
    in_maps = []
    for core in range(NCORES):
        b, g = core // 4, core % 4
        hs = g * HPC
        q_cols = w_attn[:, hs * HD:(hs + HPC) * HD]
        k_cols = w_attn[:, C + hs * HD: C + (hs + HPC) * HD]
        v_cols = w_attn[:, 2 * C + hs * HD: 2 * C + (hs + HPC) * HD]
        wqk = np.concatenate(
            [q_cols[:, 0:128], k_cols[:, 0:128], q_cols[:, 128:256], k_cols[:, 128:256]], axis=1)
        in_maps.append({
            "xT": np.ascontiguousarray(x[b].T).astype(bf16),
            "wqk": np.ascontiguousarray(wqk).astype(bf16),
            "wv": np.ascontiguousarray(v_cols).astype(bf16),
            "wp": np.ascontiguousarray(w_proj[hs * HD:(hs + HPC) * HD, :]).astype(bf16),
            "maskc": maskc,
            "sel": sel,
        })
    return in_maps


def run_cores(x, w_attn, w_proj, trace=False):
    nc = _get_nc()
    in_maps = _make_in_maps(x, w_attn, w_proj)
    res = run_bass_kernel_spmd(nc, in_maps, core_ids=list(range(NCORES)), trace=trace)
    out = np.zeros((B, T, C), dtype=np.float32)
    for core in range(NCORES):
        out[core // 4] += np.asarray(res.results[core]["out"], dtype=np.float32)
    return out, res


def kernel(x, w_attn, w_proj):
    out, _ = run_cores(x, w_attn, w_proj, trace=False)
    return out
